# revision 12
# baseline (speedup 1.0000x reference)
"""Trainium2 Bass kernel for channel-wise weighted reduction + capped relu.

Computes out[b, s] = capped_relu(sum_c x[b,c,s] * W[c,s] + bias[s]) for
x [64, 256, 4096] f32, W [256, 4096] f32, bias [4096] f32.

Sharding: S-parallel across 8 NeuronCores — core k owns s-columns
[512k, 512(k+1)) for ALL 64 batches.  Per-core HBM traffic: x 32 MiB +
W 0.5 MiB (vs 4 MiB replicated under batch sharding) + out 128 KiB, so
the DMA floor drops from ~105 us to ~96 us at the 360 GB/s per-core
aggregate.  No cross-core communication.

Per-core pipeline (64 batches as 16 groups of 4; free dim = 4 b x 512 s):
  - DMA x group-half as one SBUF tile [128ch, 4*512] (1 MiB, 2 KiB rows).
  - DVE: y_h = x_h * W_h with W read through a stride-0 broadcast AP
    (one [128,512] W tile serves all 4 batches); y tiles are dtype
    float32r — the DVE rounds, which the BIR verifier requires for f32r
    matmul inputs.
  - PE:  channel reduction as matmul, ones[128,1] f32r stationary, y
    moving.  f32r streams 1 row/cycle vs fp32's 4 => ~4x less PE busy.
    f32r matmuls may only write psum partition 0, so a group's 4x512
    outputs live on row 0, alternating bank halves (offset 2048*(g%2))
    between consecutive groups so drains overlap next group's matmuls.
    The two c-halves accumulate via start/stop into the same psum slot.
  - Drain: ACT-copy the group's psum half-row to a staging row, then
    accumulate onto out_acc[g] with an SWDGE accum DMA.  out_acc is
    preloaded with the replicated bias (host-tiled [16, 2048] input),
    fusing the bias add.
  - Epilogue on [16, 2048]: tb = out_acc ; mask = is_le(max(tb,0),1) ;
    o = max(tb,0)*mask ; store to the out[64, 512] shard.
"""

import numpy as np

B, C, S = 64, 256, 4096
NCORES = 8
SS = S // NCORES           # s-columns per core (512)
GB = 4                     # batches per group
G = B // GB                # batch groups per core (16)
H = C // 128               # channel halves
FREE = GB * SS             # free width of packed tiles (2048)

_cache = {}


def _build_nc():
    import concourse.bacc as bacc
    import concourse.mybir as mybir
    from concourse.tile import TileContext

    f32 = mybir.dt.float32
    f32r = mybir.dt.float32r
    Alu = mybir.AluOpType

    nc = bacc.Bacc(
        "TRN2",
        target_bir_lowering=False,
        debug=False,
        num_devices=NCORES,
    )

    x_d = nc.dram_tensor("x", [B, C, SS], f32, kind="ExternalInput").ap()
    w_d = nc.dram_tensor("weights", [C, SS], f32, kind="ExternalInput").ap()
    b_d = nc.dram_tensor("bias_rep", [G, FREE], f32, kind="ExternalInput").ap()
    o_d = nc.dram_tensor("out", [B, SS], f32, kind="ExternalOutput").ap()

    with TileContext(nc) as tc:
        with (
            tc.tile_pool(name="consts", bufs=1) as cpool,
            tc.tile_pool(name="xbuf", bufs=5) as xpool,
            tc.tile_pool(name="ybuf", bufs=2) as ypool,
            tc.tile_pool(name="stg", bufs=2) as spool,
            tc.tile_pool(name="epi", bufs=1) as epool,
            tc.tile_pool(name="ps", bufs=1, space="PSUM") as ppool,
        ):
            # Tiles for constants; their loads are emitted after pair 0's
            # x loads so the big stream starts immediately.
            w_t = cpool.tile([128, H * SS], f32, name="w_t")
            ones_f = cpool.tile([128, 1], f32, name="ones_f")
            ones_t = cpool.tile([128, 1], f32r, name="ones_t")
            psum_big = ppool.tile([128, S], f32, name="psum_big")
            out_acc = epool.tile([G, FREE], f32, name="out_acc")

            def load_consts():
                # W halves, once (0.5 MiB); bias preload into out_acc; f32r
                # ones (memset can't emit f32r, round through a DVE ALU op).
                for h in range(H):
                    nc.scalar.dma_start(
                        w_t[:, h * SS:(h + 1) * SS], w_d[h * 128:(h + 1) * 128, :]
                    )
                nc.vector.memset(ones_f[:], 1.0)
                nc.vector.tensor_scalar_add(ones_t[:], ones_f[:], 0.0)
                nc.scalar.dma_start(out_acc[:, :], b_d[:, :])

            # Two groups per DMA (2 MiB transfers) halves the per-transfer
            # queue bubble (post-DMA semaphore latency).  Only SP and ACT
            # have HWDGE queues, and ACT runs the psum drains, so the whole
            # x stream stays on sync.
            qeng = [nc.sync, nc.sync]
            for p in range(G // 2):
                last = p == G // 2 - 1
                xh = [
                    xpool.tile([128, 2 * FREE], f32, name=f"x_h{h}", tag=f"x{h}", bufs=3)
                    for h in range(H)
                ]
                # [128 ch, nb b, 512 s] gathers of 2 KiB rows.  The final
                # pair loads per-group (1 MiB) so only one group's compute
                # chain remains after the last transfer lands.
                loads = [(0, 2 * GB)] if not last else [(0, GB), (GB, GB)]
                for b0, nb in loads:
                    for h in range(H):
                        qeng[h].dma_start(
                            xh[h][:, b0 * SS:(b0 + nb) * SS],
                            x_d[p * 2 * GB + b0:p * 2 * GB + b0 + nb,
                                h * 128:(h + 1) * 128, :]
                            .transpose([1, 0, 2]),
                        )
                if p == 0:
                    load_consts()
                for sub in range(2):
                    g = 2 * p + sub
                    off0 = (g % 2) * FREE   # psum row-0 bank half
                    yh = [
                        ypool.tile([128, FREE], f32r, name=f"y_h{h}", tag=f"y{h}", bufs=2)
                        for h in range(H)
                    ]
                    for h in range(H):
                        # One [128, 2048] multiply per half; W chunk broadcast
                        # along the batch axis via a stride-0 AP.
                        w_b = (
                            w_t[:, h * SS:(h + 1) * SS]
                            .unsqueeze(1)
                            .broadcast_to([128, GB, SS])
                        )
                        nc.vector.tensor_tensor(
                            yh[h][:, :].rearrange("p (b s) -> p b s", b=GB),
                            xh[h][:, sub * FREE:(sub + 1) * FREE]
                            .rearrange("p (b s) -> p b s", b=GB),
                            w_b,
                            Alu.mult,
                        )
                    for j in range(GB):
                        off = off0 + j * 512
                        for h in range(H):
                            nc.tensor.matmul(
                                psum_big[0:1, off:off + 512],
                                ones_t[:, 0:1],
                                yh[h][:, j * 512:(j + 1) * 512],
                                start=(h == 0),
                                stop=(h == H - 1),
                            )
                    # Drain the group's psum half-row via ACT to a partition-0
                    # staging row, then accumulate onto out_acc[g].
                    stg = spool.tile([1, FREE], f32, name="stg", tag="stg")
                    nc.scalar.activation(
                        stg[:, :],
                        psum_big[0:1, off0:off0 + FREE],
                        mybir.ActivationFunctionType.Copy,
                    )
                    nc.gpsimd.dma_start(
                        out_acc[g:g + 1, :], stg[:, :], accum_op=Alu.add
                    )

            # Epilogue: capped relu in two free-halves, then store.  Free
            # half fh covers batch-within-group b4 in [2*fh, 2*fh+2), i.e.
            # out rows 4g + b4.
            for fh in range(2):
                sl = slice(fh * (FREE // 2), (fh + 1) * (FREE // 2))
                msk = epool.tile([G, FREE // 2], f32, name="msk", tag="msk", bufs=1)
                nc.vector.tensor_scalar(
                    msk[:], out_acc[:, sl], 0.0, 1.0, Alu.max, Alu.is_le
                )
                nc.vector.scalar_tensor_tensor(
                    out_acc[:, sl], out_acc[:, sl], 0.0, msk[:], Alu.max, Alu.mult
                )
                # dest rows b = 4g + b4, b4 in [2fh, 2fh+2)
                dst = (
                    o_d.rearrange("(g b) s -> g b s", g=G)[:, 2 * fh:2 * fh + 2, :]
                )
                nc.sync.dma_start(
                    dst,
                    out_acc[:, sl].rearrange("p (b s) -> p b s", b=GB // 2),
                )

    nc.compile()
    return nc


def shard_inputs(x, weights, bias):
    """Per-core input shards for S-parallel layout."""
    x = np.ascontiguousarray(x, dtype=np.float32)
    weights = np.ascontiguousarray(weights, dtype=np.float32)
    bias = np.ascontiguousarray(bias, dtype=np.float32)
    maps = []
    for k in range(NCORES):
        sl = slice(k * SS, (k + 1) * SS)
        maps.append(
            {
                "x": np.ascontiguousarray(x[:, :, sl]),
                "weights": np.ascontiguousarray(weights[:, sl]),
                "bias_rep": np.tile(bias[sl], (G, GB)).astype(np.float32),
            }
        )
    return maps


def kernel(x: np.ndarray, weights: np.ndarray, bias: np.ndarray) -> np.ndarray:
    from concourse.bass_utils import run_bass_kernel_spmd

    if "nc" not in _cache:
        _cache["nc"] = _build_nc()
    nc = _cache["nc"]

    in_maps = shard_inputs(x, weights, bias)
    res = run_bass_kernel_spmd(nc, in_maps, core_ids=list(range(NCORES)))
    out = np.empty((B, S), dtype=np.float32)
    for k in range(NCORES):
        out[:, k * SS:(k + 1) * SS] = res.results[k]["out"]
    return out


# revision 14
# speedup vs baseline: 1.0726x; 1.0726x over previous
"""Trainium2 Bass kernel for channel-wise weighted reduction + capped relu.

Computes out[b, s] = capped_relu(sum_c x[b,c,s] * W[c,s] + bias[s]) for
x [64, 256, 4096] f32, W [256, 4096] f32, bias [4096] f32.

Sharding: S-parallel across 8 NeuronCores — core k owns s-columns
[512k, 512(k+1)) for ALL 64 batches.  Per-core HBM traffic: x 32 MiB +
W 0.5 MiB (vs 4 MiB replicated under batch sharding) + out 128 KiB, so
the DMA floor drops from ~105 us to ~96 us at the 360 GB/s per-core
aggregate.  No cross-core communication.

Per-core pipeline (64 batches as 16 groups of 4; free dim = 4 b x 512 s):
  - DMA a PAIR of groups per c-half as one SBUF tile [128ch, 8*512]
    (2 MiB, 2 KiB rows) — big transfers halve the per-DMA queue bubble
    and the measured stream is gapless at the chip HBM roofline.  The
    final pair loads per-group so only one group's compute chain remains
    after the last transfer lands.
  - DVE: y_h = x_h * W_h with W read through a stride-0 broadcast AP
    (one [128,512] W tile serves all 4 batches); y tiles are dtype
    float32r — the DVE rounds, which the BIR verifier requires for f32r
    matmul inputs.
  - PE:  channel reduction as matmul, ones[128,1] f32r stationary, y
    moving.  f32r streams 1 row/cycle vs fp32's 4 => ~4x less PE busy.
    f32r matmuls may only write psum partition 0, so a group's 4x512
    outputs live on row 0, alternating bank halves (offset 2048*(g%2))
    between consecutive groups so drains overlap next group's matmuls.
    The two c-halves accumulate via start/stop into the same psum slot.
  - Drain: ACT-copy the group's psum half-row to a staging row, then
    accumulate onto out_acc[g] with an SWDGE accum DMA.  out_acc is
    preloaded with the replicated bias (host-tiled [16, 2048] input),
    fusing the bias add.
  - Epilogue on [16, 2048]: tb = out_acc ; mask = is_le(max(tb,0),1) ;
    o = max(tb,0)*mask ; store to the out[64, 512] shard.
"""

import numpy as np

B, C, S = 64, 256, 4096
NCORES = 8
SS = S // NCORES           # s-columns per core (512)
GB = 4                     # batches per group
G = B // GB                # batch groups per core (16)
H = C // 128               # channel halves
FREE = GB * SS             # free width of packed tiles (2048)

_cache = {}


def _build_nc():
    import concourse.bacc as bacc
    import concourse.mybir as mybir
    from concourse.tile import TileContext

    f32 = mybir.dt.float32
    f32r = mybir.dt.float32r
    Alu = mybir.AluOpType

    nc = bacc.Bacc(
        "TRN2",
        target_bir_lowering=False,
        debug=False,
        num_devices=NCORES,
    )

    x_d = nc.dram_tensor("x", [B, C, SS], f32, kind="ExternalInput").ap()
    w_d = nc.dram_tensor("weights", [C, SS], f32, kind="ExternalInput").ap()
    b_d = nc.dram_tensor("bias_rep", [G, FREE], f32, kind="ExternalInput").ap()
    o_d = nc.dram_tensor("out", [B, SS], f32, kind="ExternalOutput").ap()

    with TileContext(nc) as tc:
        with (
            tc.tile_pool(name="consts", bufs=1) as cpool,
            tc.tile_pool(name="xbuf", bufs=5) as xpool,
            tc.tile_pool(name="ybuf", bufs=2) as ypool,
            tc.tile_pool(name="stg", bufs=2) as spool,
            tc.tile_pool(name="epi", bufs=1) as epool,
            tc.tile_pool(name="ps", bufs=1, space="PSUM") as ppool,
        ):
            # Tiles for constants; their loads are emitted after pair 0's
            # x loads so the big stream starts immediately.
            w_t = cpool.tile([128, H * SS], f32, name="w_t")
            ones_f = cpool.tile([128, 1], f32, name="ones_f")
            ones_t = cpool.tile([128, 1], f32r, name="ones_t")
            psum_big = ppool.tile([128, S], f32, name="psum_big")
            out_acc = epool.tile([G, FREE], f32, name="out_acc")

            def load_consts():
                # W halves, once (0.5 MiB); bias preload into out_acc; f32r
                # ones (memset can't emit f32r, round through a DVE ALU op).
                for h in range(H):
                    nc.scalar.dma_start(
                        w_t[:, h * SS:(h + 1) * SS], w_d[h * 128:(h + 1) * 128, :]
                    )
                nc.vector.memset(ones_f[:], 1.0)
                nc.vector.tensor_scalar_add(ones_t[:], ones_f[:], 0.0)
                nc.scalar.dma_start(out_acc[:, :], b_d[:, :])

            # Two groups per DMA (2 MiB transfers) halves the per-transfer
            # queue bubble (post-DMA semaphore latency).  Only SP and ACT
            # have HWDGE queues, and ACT runs the psum drains, so the whole
            # x stream stays on sync.
            qeng = [nc.sync, nc.sync]
            for p in range(G // 2):
                last = p == G // 2 - 1
                xh = [
                    xpool.tile([128, 2 * FREE], f32, name=f"x_h{h}", tag=f"x{h}", bufs=3)
                    for h in range(H)
                ]
                # [128 ch, nb b, 512 s] gathers of 2 KiB rows.  The final
                # pair loads per-group (1 MiB) so only one group's compute
                # chain remains after the last transfer lands.
                loads = [(0, 2 * GB)] if not last else [(0, GB), (GB, GB)]
                for b0, nb in loads:
                    for h in range(H):
                        qeng[h].dma_start(
                            xh[h][:, b0 * SS:(b0 + nb) * SS],
                            x_d[p * 2 * GB + b0:p * 2 * GB + b0 + nb,
                                h * 128:(h + 1) * 128, :]
                            .transpose([1, 0, 2]),
                        )
                if p == 0:
                    load_consts()
                for sub in range(2):
                    g = 2 * p + sub
                    off0 = (g % 2) * FREE   # psum row-0 bank half
                    tail = last and sub == 1
                    yh = [
                        ypool.tile([128, FREE], f32r, name=f"y_h{h}", tag=f"y{h}", bufs=2)
                        for h in range(H)
                    ]
                    # Multiply chunking: NQ=1 normally (one [128, 2048] op per
                    # half, W broadcast along batches via stride-0 AP).  The
                    # final group's h1 half is the post-stream critical path,
                    # so it runs per 512-chunk with matmuls and chunked ACT
                    # drains pipelined behind the DVE stream.
                    for h in range(H):
                        nq = GB if (tail and h == H - 1) else 1
                        cw = FREE // nq
                        nb = GB // nq
                        for q in range(nq):
                            w_b = w_t[:, h * SS:(h + 1) * SS]
                            if nb > 1:
                                w_b = w_b.unsqueeze(1).broadcast_to([128, nb, SS])
                            ysl = yh[h][:, q * cw:(q + 1) * cw]
                            xsl = xh[h][:, sub * FREE + q * cw:sub * FREE + (q + 1) * cw]
                            if nb > 1:
                                ysl = ysl.rearrange("p (b s) -> p b s", b=nb)
                                xsl = xsl.rearrange("p (b s) -> p b s", b=nb)
                            nc.vector.tensor_tensor(ysl, xsl, w_b, Alu.mult)
                    stg = spool.tile([1, FREE], f32, name="stg", tag="stg")
                    for j in range(GB):
                        off = off0 + j * 512
                        for h in range(H):
                            nc.tensor.matmul(
                                psum_big[0:1, off:off + 512],
                                ones_t[:, 0:1],
                                yh[h][:, j * 512:(j + 1) * 512],
                                start=(h == 0),
                                stop=(h == H - 1),
                            )
                        if tail:
                            # per-chunk drain right behind the chunk's stop
                            nc.scalar.activation(
                                stg[:, j * 512:(j + 1) * 512],
                                psum_big[0:1, off:off + 512],
                                mybir.ActivationFunctionType.Copy,
                            )
                            if j % 2 == 1:
                                hsl = slice((j - 1) * 512, (j + 1) * 512)
                                nc.gpsimd.dma_start(
                                    out_acc[g:g + 1, hsl], stg[:, hsl],
                                    accum_op=Alu.add,
                                )
                    if not tail:
                        # Drain the group's psum half-row via ACT to a
                        # partition-0 staging row, then accumulate onto
                        # out_acc[g] (bias already there).
                        nc.scalar.activation(
                            stg[:, :],
                            psum_big[0:1, off0:off0 + FREE],
                            mybir.ActivationFunctionType.Copy,
                        )
                        nc.gpsimd.dma_start(
                            out_acc[g:g + 1, :], stg[:, :], accum_op=Alu.add
                        )

            # Epilogue: capped relu in two free-halves, then store.  Free
            # half fh covers batch-within-group b4 in [2*fh, 2*fh+2), i.e.
            # out rows 4g + b4.
            for fh in range(2):
                sl = slice(fh * (FREE // 2), (fh + 1) * (FREE // 2))
                msk = epool.tile([G, FREE // 2], f32, name="msk", tag="msk", bufs=1)
                nc.vector.tensor_scalar(
                    msk[:], out_acc[:, sl], 0.0, 1.0, Alu.max, Alu.is_le
                )
                nc.vector.scalar_tensor_tensor(
                    out_acc[:, sl], out_acc[:, sl], 0.0, msk[:], Alu.max, Alu.mult
                )
                # dest rows b = 4g + b4, b4 in [2fh, 2fh+2)
                dst = (
                    o_d.rearrange("(g b) s -> g b s", g=G)[:, 2 * fh:2 * fh + 2, :]
                )
                nc.sync.dma_start(
                    dst,
                    out_acc[:, sl].rearrange("p (b s) -> p b s", b=GB // 2),
                )

    nc.compile()
    return nc


def shard_inputs(x, weights, bias):
    """Per-core input shards for S-parallel layout."""
    x = np.ascontiguousarray(x, dtype=np.float32)
    weights = np.ascontiguousarray(weights, dtype=np.float32)
    bias = np.ascontiguousarray(bias, dtype=np.float32)
    maps = []
    for k in range(NCORES):
        sl = slice(k * SS, (k + 1) * SS)
        maps.append(
            {
                "x": np.ascontiguousarray(x[:, :, sl]),
                "weights": np.ascontiguousarray(weights[:, sl]),
                "bias_rep": np.tile(bias[sl], (G, GB)).astype(np.float32),
            }
        )
    return maps


def kernel(x: np.ndarray, weights: np.ndarray, bias: np.ndarray) -> np.ndarray:
    from concourse.bass_utils import run_bass_kernel_spmd

    if "nc" not in _cache:
        _cache["nc"] = _build_nc()
    nc = _cache["nc"]

    in_maps = shard_inputs(x, weights, bias)
    res = run_bass_kernel_spmd(nc, in_maps, core_ids=list(range(NCORES)))
    out = np.empty((B, S), dtype=np.float32)
    for k in range(NCORES):
        out[:, k * SS:(k + 1) * SS] = res.results[k]["out"]
    return out


# revision 25
# speedup vs baseline: 1.1308x; 1.0542x over previous
"""Trainium2 Bass kernel for channel-wise weighted reduction + capped relu.

Computes out[b, s] = capped_relu(sum_c x[b,c,s] * W[c,s] + bias[s]) for
x [64, 256, 4096] f32, W [256, 4096] f32, bias [4096] f32.

Sharding: S-parallel across 8 NeuronCores — core k owns s-columns
[512k, 512(k+1)) for ALL 64 batches.  Per-core HBM traffic: x 32 MiB +
W 0.5 MiB (vs 4 MiB replicated under batch sharding) + out 128 KiB, so
the DMA floor drops from ~105 us to ~96 us at the 360 GB/s per-core
aggregate.  No cross-core communication.

Per-core pipeline (64 batches as 16 groups of 4; free dim = 4 b x 512 s):
  - DMA a PAIR of groups per c-half as one SBUF tile [128ch, 8*512]
    (2 MiB, 2 KiB rows) — big transfers halve the per-DMA queue bubble
    and the measured stream is gapless at the chip HBM roofline.  The
    final pair loads per-group so only one group's compute chain remains
    after the last transfer lands.
  - DVE: y_h = x_h * W_h with W read through a stride-0 broadcast AP
    (one [128,512] W tile serves all 4 batches); y tiles are dtype
    float32r — the DVE rounds, which the BIR verifier requires for f32r
    matmul inputs.
  - PE:  channel reduction as matmul, ones[128,1] f32r stationary, y
    moving.  f32r streams 1 row/cycle vs fp32's 4 => ~4x less PE busy.
    f32r matmuls may only write psum partition 0, so a group's 4x512
    outputs live on row 0, alternating bank halves (offset 2048*(g%2))
    between consecutive groups so drains overlap next group's matmuls.
    The two c-halves accumulate via start/stop into the same psum slot.
  - Drain: ACT-copy the group's psum half-row to a staging row, then
    accumulate onto out_acc[g] with an SWDGE accum DMA.  out_acc is
    preloaded with the replicated bias (host-tiled [16, 2048] input),
    fusing the bias add.
  - Epilogue on [16, 2048]: r = relu(out_acc) on the idle ACT engine,
    then one DVE op o = (r <= 1)*r per free-half; store the out[64, 512]
    shard.  Precision note: f32r product rounding costs ~2e-4 max abs
    error; anything coarser (fp16/bf16 x ingestion) flips outputs across
    the capped-relu threshold at 1.0 (nearest preactivation is 1.2e-4
    from the cap) and fails the max-error gate.
"""

import numpy as np

B, C, S = 64, 256, 4096
NCORES = 8
SS = S // NCORES           # s-columns per core (512)
GB = 4                     # batches per group
G = B // GB                # batch groups per core (16)
H = C // 128               # channel halves
FREE = GB * SS             # free width of packed tiles (2048)

_cache = {}


def _build_nc():
    import concourse.bacc as bacc
    import concourse.mybir as mybir
    from concourse.tile import TileContext

    f32 = mybir.dt.float32
    f32r = mybir.dt.float32r
    Alu = mybir.AluOpType

    nc = bacc.Bacc(
        "TRN2",
        target_bir_lowering=False,
        debug=False,
        num_devices=NCORES,
    )

    x_d = nc.dram_tensor("x", [B, C, SS], f32, kind="ExternalInput").ap()
    w_d = nc.dram_tensor("weights", [C, SS], f32, kind="ExternalInput").ap()
    b_d = nc.dram_tensor("bias_rep", [G, FREE], f32, kind="ExternalInput").ap()
    o_d = nc.dram_tensor("out", [B, SS], f32, kind="ExternalOutput").ap()

    with TileContext(nc) as tc:
        with (
            tc.tile_pool(name="consts", bufs=1) as cpool,
            tc.tile_pool(name="xbuf", bufs=5) as xpool,
            tc.tile_pool(name="ybuf", bufs=2) as ypool,
            tc.tile_pool(name="stg", bufs=2) as spool,
            tc.tile_pool(name="epi", bufs=1) as epool,
            tc.tile_pool(name="ps", bufs=1, space="PSUM") as ppool,
        ):
            # Tiles for constants; their loads are emitted after pair 0's
            # x loads so the big stream starts immediately.
            w_t = cpool.tile([128, H * SS], f32, name="w_t")
            ones_f = cpool.tile([128, 1], f32, name="ones_f")
            ones_t = cpool.tile([128, 1], f32r, name="ones_t")
            psum_big = ppool.tile([128, S], f32, name="psum_big")
            out_acc = epool.tile([G, FREE], f32, name="out_acc")
            bias_row = cpool.tile([1, FREE], f32, name="bias_row")
            bias_r = cpool.tile([1, FREE], f32r, name="bias_r")

            def load_consts():
                # W halves, once (0.5 MiB); bias row for psum preloads; f32r
                # ones (memset can't emit f32r, round through a DVE ALU op).
                for h in range(H):
                    nc.scalar.dma_start(
                        w_t[:, h * SS:(h + 1) * SS], w_d[h * 128:(h + 1) * 128, :]
                    )
                nc.vector.memset(ones_f[:], 1.0)
                nc.vector.tensor_scalar_add(ones_t[:], ones_f[:], 0.0)
                nc.scalar.dma_start(bias_row[:, :], b_d[0:1, :])
                nc.vector.tensor_scalar_add(bias_r[:, :], bias_row[:, :], 0.0)

            # Two groups per DMA (2 MiB transfers) halves the per-transfer
            # queue bubble (post-DMA semaphore latency).  Only SP and ACT
            # have HWDGE queues, and ACT runs the psum drains, so the whole
            # x stream stays on sync.
            qeng = [nc.sync, nc.sync]
            for p in range(G // 2):
                last = p == G // 2 - 1
                xh = [
                    xpool.tile([128, 2 * FREE], f32, name=f"x_h{h}", tag=f"x{h}", bufs=3)
                    for h in range(H)
                ]
                # [128 ch, nb b, 512 s] gathers of 2 KiB rows.  The final
                # pair loads per-group (1 MiB) so only one group's compute
                # chain remains after the last transfer lands.
                loads = [(0, 2 * GB)] if not last else [(0, GB), (GB, GB)]
                for b0, nb in loads:
                    for h in range(H):
                        qeng[h].dma_start(
                            xh[h][:, b0 * SS:(b0 + nb) * SS],
                            x_d[p * 2 * GB + b0:p * 2 * GB + b0 + nb,
                                h * 128:(h + 1) * 128, :]
                            .transpose([1, 0, 2]),
                        )
                if p == 0:
                    load_consts()
                for sub in range(2):
                    g = 2 * p + sub
                    off0 = (g % 2) * FREE   # psum row-0 bank half
                    tail = last and sub == 1
                    yh = [
                        ypool.tile([128, FREE], f32r, name=f"y_h{h}", tag=f"y{h}", bufs=2)
                        for h in range(H)
                    ]
                    # The bias enters each psum chunk as the accumulation
                    # group's STARTING matmul (K=1: ones[1,1]^T @ bias[1,512])
                    # so the slot holds tb = bias + sums when the group
                    # finishes and the drain can apply Relu directly.
                    # Multiply chunking: NQ=1 normally (one [128, 2048] op per
                    # half, W broadcast along batches via stride-0 AP).  The
                    # final group's h1 half is the post-stream critical path,
                    # so it runs per 512-chunk with matmuls and chunked ACT
                    # drains pipelined behind the DVE stream.
                    for h in range(H):
                        nq = GB if (tail and h == H - 1) else 1
                        cw = FREE // nq
                        nb = GB // nq
                        for q in range(nq):
                            w_b = w_t[:, h * SS:(h + 1) * SS]
                            if nb > 1:
                                w_b = w_b.unsqueeze(1).broadcast_to([128, nb, SS])
                            ysl = yh[h][:, q * cw:(q + 1) * cw]
                            xsl = xh[h][:, sub * FREE + q * cw:sub * FREE + (q + 1) * cw]
                            if nb > 1:
                                ysl = ysl.rearrange("p (b s) -> p b s", b=nb)
                                xsl = xsl.rearrange("p (b s) -> p b s", b=nb)
                            nc.vector.tensor_tensor(ysl, xsl, w_b, Alu.mult)
                    stg = spool.tile([1, FREE], f32, name="stg", tag="stg")
                    for j in range(GB):
                        off = off0 + j * 512
                        nc.tensor.matmul(
                            psum_big[0:1, off:off + 512],
                            ones_t[0:1, 0:1],
                            bias_r[:, j * 512:(j + 1) * 512],
                            start=True,
                            stop=False,
                        )
                        for h in range(H):
                            nc.tensor.matmul(
                                psum_big[0:1, off:off + 512],
                                ones_t[:, 0:1],
                                yh[h][:, j * 512:(j + 1) * 512],
                                start=False,
                                stop=(h == H - 1),
                            )
                        if tail:
                            # per-chunk relu-drain right behind the chunk's
                            # stop; pack halves ride the same scalar queue
                            nc.scalar.activation(
                                stg[:, j * 512:(j + 1) * 512],
                                psum_big[0:1, off:off + 512],
                                mybir.ActivationFunctionType.Relu,
                            )
                            if j % 2 == 1:
                                hsl = slice((j - 1) * 512, (j + 1) * 512)
                                nc.scalar.dma_start(
                                    out_acc[g:g + 1, hsl], stg[:, hsl]
                                )
                    if not tail:
                        # Relu-drain the group's psum half-row via ACT to a
                        # partition-0 staging row (psum already holds
                        # bias + sums), then pack onto out_acc[g] with a
                        # plain DMA on the same queue.
                        nc.scalar.activation(
                            stg[:, :],
                            psum_big[0:1, off0:off0 + FREE],
                            mybir.ActivationFunctionType.Relu,
                        )
                        nc.scalar.dma_start(out_acc[g:g + 1, :], stg[:, :])

            # Epilogue: out_acc rows already hold r = relu(bias + sums), so
            # the cap needs ONE in-place DVE op per free-half:
            # out = (r <= 1) * r.  Free half fh covers batch-within-group
            # b4 in [2*fh, 2*fh+2), i.e. out rows 4g + b4.
            for fh in range(2):
                sl = slice(fh * (FREE // 2), (fh + 1) * (FREE // 2))
                nc.vector.scalar_tensor_tensor(
                    out_acc[:, sl], out_acc[:, sl], 1.0, out_acc[:, sl],
                    Alu.is_le, Alu.mult,
                )
                # dest rows b = 4g + b4, b4 in [2fh, 2fh+2)
                dst = (
                    o_d.rearrange("(g b) s -> g b s", g=G)[:, 2 * fh:2 * fh + 2, :]
                )
                nc.sync.dma_start(
                    dst,
                    out_acc[:, sl].rearrange("p (b s) -> p b s", b=GB // 2),
                )

    nc.compile()
    return nc


def shard_inputs(x, weights, bias):
    """Per-core input shards for S-parallel layout."""
    x = np.ascontiguousarray(x, dtype=np.float32)
    weights = np.ascontiguousarray(weights, dtype=np.float32)
    bias = np.ascontiguousarray(bias, dtype=np.float32)
    maps = []
    for k in range(NCORES):
        sl = slice(k * SS, (k + 1) * SS)
        maps.append(
            {
                "x": np.ascontiguousarray(x[:, :, sl]),
                "weights": np.ascontiguousarray(weights[:, sl]),
                "bias_rep": np.tile(bias[sl], (G, GB)).astype(np.float32),
            }
        )
    return maps


def kernel(x: np.ndarray, weights: np.ndarray, bias: np.ndarray) -> np.ndarray:
    from concourse.bass_utils import run_bass_kernel_spmd

    if "nc" not in _cache:
        _cache["nc"] = _build_nc()
    nc = _cache["nc"]

    in_maps = shard_inputs(x, weights, bias)
    res = run_bass_kernel_spmd(nc, in_maps, core_ids=list(range(NCORES)))
    out = np.empty((B, S), dtype=np.float32)
    for k in range(NCORES):
        out[:, k * SS:(k + 1) * SS] = res.results[k]["out"]
    return out
